# revision 9
# baseline (speedup 1.0000x reference)
"""Multi-head attention (B=2, S=2048, D=1024, H=16) on 8 TRN2 NeuronCores.

Sharding: data-parallel over batch (2) x tensor-parallel over head groups (4).
Core c handles batch c//4, heads 4*(c%4) .. 4*(c%4)+3 (256 projection dims).
Each core computes its partial output projection; the host sums the 4 partials
per batch and adds the (bv @ wo.T + bo) constant, which is exact because
softmax weights sum to 1.

Device layouts (per core):
  QT/KT  [128, 2, 2048] bf16 : partition p + 128*m = local proj dim, free = seq
  V_sb   [128, 16, 4, 65] bf16: [k-pos within tile, k-tile, head, dk + ones col]
  scores computed transposed: ST[k, q] = K'h @ Q'h^T, exp on ScalarE,
  PV: attnT[dk, q] += [Vh | 1]^T @ expST  (ones column yields softmax denom)
  out-proj: out[s, dout] = attnT^T @ woT, partial, f32 to DRAM.
"""

import sys

sys.path.insert(0, "/opt/trn_rl_repo")

import numpy as np
import ml_dtypes

BF16 = ml_dtypes.bfloat16

B, S, D = 2, 2048, 1024
H, DK = 16, 64
N_CORES = 8
GROUPS = 4  # head groups (tensor-parallel)
DL = D // GROUPS  # 256 local projection dims per core
SCALE = 1.0 / np.sqrt(np.sqrt(float(DK)))  # fold 1/sqrt(dk) half into Q, half into K

_cache: dict = {}


def _build():
    import concourse.mybir as mybir
    import concourse.tile as tile
    from concourse import bacc

    dt = mybir.dt
    f32, bf16 = dt.float32, dt.bfloat16

    nc = bacc.Bacc("TRN2", target_bir_lowering=False, debug=False,
                   num_devices=N_CORES)

    xqT = nc.dram_tensor("xqT", [D, S], bf16, kind="ExternalInput").ap()
    xkT = nc.dram_tensor("xkT", [D, S], bf16, kind="ExternalInput").ap()
    xvT = nc.dram_tensor("xvT", [D, S], bf16, kind="ExternalInput").ap()
    wqT = nc.dram_tensor("wqT", [D, DL], bf16, kind="ExternalInput").ap()
    wkT = nc.dram_tensor("wkT", [D, DL], bf16, kind="ExternalInput").ap()
    wvT = nc.dram_tensor("wvT", [D, DL], bf16, kind="ExternalInput").ap()
    woT = nc.dram_tensor("woT", [DL, D], bf16, kind="ExternalInput").ap()
    bqk = nc.dram_tensor("bqk", [2, DL], f32, kind="ExternalInput").ap()
    out = nc.dram_tensor("out", [S, D], f32, kind="ExternalOutput").ap()

    EXPF = mybir.ActivationFunctionType.Exp

    with tile.TileContext(nc) as tc:
        with (
            tc.tile_pool(name="res", bufs=1) as res,
            tc.tile_pool(name="wts", bufs=1) as wts,
            tc.tile_pool(name="xin", bufs=2) as xin,
            tc.tile_pool(name="expp", bufs=3) as expp,
            tc.tile_pool(name="nrm", bufs=2) as nrm,
            tc.tile_pool(name="drm", bufs=2, space="DRAM") as drm,
            tc.tile_pool(name="ps_proj", bufs=2, space="PSUM") as ps_proj,
            tc.tile_pool(name="ps_st", bufs=2, space="PSUM") as ps_st,
            tc.tile_pool(name="ps_at", bufs=2, space="PSUM") as ps_at,
        ):
            # ---- resident tensors ----
            QT = res.tile([128, 2, S], bf16)
            KT = res.tile([128, 2, S], bf16)
            Vsb = res.tile([128, 16, 4, DK + 1], bf16)
            ATT = res.tile([128, 2, S], bf16)

            wq_sb = wts.tile([128, 8, DL], bf16, tag="wq")
            wk_sb = wts.tile([128, 8, DL], bf16, tag="wk")
            wv_sb = wts.tile([128, 8, DL], bf16, tag="wv")
            wo_sb = wts.tile([128, 2, D], bf16, tag="wo")
            b_sb = wts.tile([128, 2, 2], f32, tag="b")  # [p, proj(q/k), m]

            for kt in range(8):
                nc.sync.dma_start(wq_sb[:, kt, :], wqT[kt * 128:(kt + 1) * 128, :])
                nc.sync.dma_start(wk_sb[:, kt, :], wkT[kt * 128:(kt + 1) * 128, :])
                nc.sync.dma_start(wv_sb[:, kt, :], wvT[kt * 128:(kt + 1) * 128, :])
            for m in range(2):
                nc.sync.dma_start(wo_sb[:, m, :], woT[m * 128:(m + 1) * 128, :])
                for pj in range(2):
                    nc.sync.dma_start(b_sb[:, pj, m:m + 1],
                                      bqk[pj, m * 128:(m + 1) * 128, None])
            # ones column for softmax denominators
            nc.vector.memset(Vsb[:, :, :, DK], 1.0)

            # ---- Phase A: Q, K projections -> QT/KT [dout, s] ----
            for pj, (xsrc, wsb, dst) in enumerate(
                    [(xqT, wq_sb, QT), (xkT, wk_sb, KT)]):
                for sb in range(4):
                    xb = xin.tile([128, 8, 512], bf16, tag="xblk")
                    nc.sync.dma_start(
                        xb[:, :, :],
                        xsrc[:, sb * 512:(sb + 1) * 512].rearrange(
                            "(n p) d -> p n d", p=128))
                    for m in range(2):
                        ps = ps_proj.tile([128, 512], f32, tag="proj")
                        for kt in range(8):
                            nc.tensor.matmul(
                                ps[:, :],
                                lhsT=wsb[:, kt, m * 128:(m + 1) * 128],
                                rhs=xb[:, kt, :],
                                start=(kt == 0), stop=(kt == 7))
                        nc.vector.tensor_scalar_add(
                            dst[:, m, sb * 512:(sb + 1) * 512],
                            ps[:, :], b_sb[:, pj, m:m + 1])

            # ---- Phase B: V projection -> Vsb [kpos, ktile, head, dk] ----
            for stq in range(4):
                xb = xin.tile([128, 8, 512], bf16, tag="xblk")
                nc.sync.dma_start(
                    xb[:, :, :],
                    xvT[:, stq * 512:(stq + 1) * 512].rearrange(
                        "(n p) d -> p n d", p=128))
                for sts in range(4):
                    st = stq * 4 + sts
                    ps = ps_proj.tile([128, 4, DK], f32, tag="proj")
                    for kt in range(8):
                        nc.tensor.matmul(
                            ps[:, :, :],
                            lhsT=xb[:, kt, sts * 128:(sts + 1) * 128],
                            rhs=wv_sb[:, kt, :],
                            start=(kt == 0), stop=(kt == 7))
                    nc.vector.tensor_copy(Vsb[:, st, :, 0:DK], ps[:, :, :])

            # ---- Phase C: attention per head pair / q block ----
            for hp in range(2):  # head pair: local heads (2*hp, 2*hp+1)
                for qb in range(4):  # q blocks of 512
                    qs = slice(qb * 512, (qb + 1) * 512)
                    pa = [ps_at.tile([65, 512], f32, tag="at", name=f"at{i}")
                          for i in range(2)]
                    for p in range(8):  # k-tile pairs (kt = 2p, 2p+1)
                        for hh in range(2):  # head within pair
                            lh = 2 * hp + hh
                            lo, hi = hh * 64, hh * 64 + 64
                            st_ps = ps_st.tile([128, 2, 512], f32, tag="st")
                            for j in range(2):
                                kt = 2 * p + j
                                nc.tensor.matmul(
                                    st_ps[:, j, :],
                                    lhsT=KT[lo:hi, hp, kt * 128:(kt + 1) * 128],
                                    rhs=QT[lo:hi, hp, qs],
                                    start=True, stop=True)
                            ex = expp.tile([128, 2, 512], bf16, tag="exp")
                            nc.scalar.activation(ex[:, :, :], st_ps[:, :, :], EXPF)
                            for j in range(2):
                                kt = 2 * p + j
                                nc.tensor.matmul(
                                    pa[hh][:, :],
                                    lhsT=Vsb[:, kt, lh, :],
                                    rhs=ex[:, j, :],
                                    start=(kt == 0), stop=(kt == 15),
                                    skip_group_check=True)
                    # normalize: rows 0..63 / row 64, write into ATT
                    for hh in range(2):
                        rc = nrm.tile([128, 512], f32, tag="rc")
                        rb = nrm.tile([64, 512], f32, tag="rb")
                        nc.vector.reciprocal(rc[64:65, :], pa[hh][64:65, :])
                        rdr = drm.tile([1, 512], f32, tag="rdr")
                        nc.sync.dma_start(rdr[:, :], rc[64:65, :])
                        nc.sync.dma_start(rb[:, :],
                                          rdr[:, :].to_broadcast((64, 512)))
                        if hh == 0:
                            nc.vector.tensor_mul(ATT[0:64, hp, qs],
                                                 pa[hh][0:64, :], rb[:, :])
                        else:
                            tmp = nrm.tile([64, 512], bf16, tag="tmp")
                            nc.vector.tensor_mul(tmp[:, :], pa[hh][0:64, :],
                                                 rb[:, :])
                            nc.sync.dma_start(ATT[64:128, hp, qs], tmp[:, :])

            # ---- Phase D: output projection (partial) ----
            for st in range(16):
                for db in range(2):
                    ps = ps_proj.tile([128, 512], f32, tag="proj")
                    for m in range(2):
                        nc.tensor.matmul(
                            ps[:, :],
                            lhsT=ATT[:, m, st * 128:(st + 1) * 128],
                            rhs=wo_sb[:, m, db * 512:(db + 1) * 512],
                            start=(m == 0), stop=(m == 1))
                    osb = nrm.tile([128, 512], f32, tag="osb")
                    nc.vector.tensor_copy(osb[:, :], ps[:, :])
                    nc.sync.dma_start(
                        out[st * 128:(st + 1) * 128, db * 512:(db + 1) * 512],
                        osb[:, :])

    nc.compile()
    return nc


def _prep_inputs(q, k, v, wq, bq, wk, bk, wv, bv, wo, bo):
    q, k, v = (np.asarray(a, np.float32) for a in (q, k, v))
    wq, bq, wk, bk, wv, bv, wo, bo = (
        np.asarray(a, np.float32) for a in (wq, bq, wk, bk, wv, bv, wo, bo))

    xT = {}
    for b in range(B):
        xT[("q", b)] = np.ascontiguousarray(q[b].T).astype(BF16)
        xT[("k", b)] = np.ascontiguousarray(k[b].T).astype(BF16)
        xT[("v", b)] = np.ascontiguousarray(v[b].T).astype(BF16)

    grp = {}
    for g in range(GROUPS):
        hs = slice(g * DL, (g + 1) * DL)
        grp[g] = {
            "wqT": np.ascontiguousarray((wq[hs, :] * SCALE).T).astype(BF16),
            "wkT": np.ascontiguousarray((wk[hs, :] * SCALE).T).astype(BF16),
            "wvT": np.ascontiguousarray(wv[hs, :].T).astype(BF16),
            "woT": np.ascontiguousarray(wo[:, hs].T).astype(BF16),
            "bqk": np.stack([bq[hs] * SCALE, bk[hs] * SCALE]).astype(np.float32),
        }

    in_maps = []
    for c in range(N_CORES):
        b, g = c // GROUPS, c % GROUPS
        m = {"xqT": xT[("q", b)], "xkT": xT[("k", b)], "xvT": xT[("v", b)]}
        m.update(grp[g])
        in_maps.append(m)

    const = (bv @ wo.T + bo).astype(np.float32)  # exact since sum(P) == 1
    return in_maps, const


def _run(in_maps, trace=False):
    from concourse.bass_utils import run_bass_kernel_spmd

    if "nc" not in _cache:
        _cache["nc"] = _build()
    return run_bass_kernel_spmd(_cache["nc"], in_maps, list(range(N_CORES)),
                                trace=trace)


def _reduce(results, const):
    out = np.zeros((B, S, D), np.float32)
    for c in range(N_CORES):
        out[c // GROUPS] += results[c]["out"]
    out += const
    return out


def kernel(**inputs) -> np.ndarray:
    in_maps, const = _prep_inputs(**inputs)
    res = _run(in_maps, trace=False)
    return _reduce(res.results, const)


def kernel_profiled(**inputs):
    """Returns (output, exec_time_ns or None)."""
    in_maps, const = _prep_inputs(**inputs)
    res = _run(in_maps, trace=True)
    return _reduce(res.results, const), res.exec_time_ns


# revision 13
# speedup vs baseline: 1.1214x; 1.1214x over previous
"""Multi-head attention (B=2, S=2048, D=1024, H=16) on 8 TRN2 NeuronCores.

Sharding: data-parallel over batch (2) x tensor-parallel over head groups (4).
Core c handles batch c//4, heads 4*(c%4) .. 4*(c%4)+3 (256 projection dims).
Each core computes its partial output projection; the host sums the 4 partials
per batch and adds the (bv @ wo.T + bo) constant, which is exact because
softmax weights sum to 1.

Device layouts (per core):
  QT/KT  [128, 2, 2048] bf16 : partition p + 128*m = local proj dim, free = seq
  V_sb   [128, 16, 4, 65] bf16: [k-pos within tile, k-tile, head, dk + ones col]
  scores computed transposed: ST[k, q] = K'h @ Q'h^T, exp on ScalarE,
  PV: attnT[dk, q] += [Vh | 1]^T @ expST  (ones column yields softmax denom)
  out-proj: out[s, dout] = attnT^T @ woT, partial, f32 to DRAM.
"""

import sys

sys.path.insert(0, "/opt/trn_rl_repo")

import numpy as np
import ml_dtypes

BF16 = ml_dtypes.bfloat16

B, S, D = 2, 2048, 1024
H, DK = 16, 64
N_CORES = 8
GROUPS = 4  # head groups (tensor-parallel)
DL = D // GROUPS  # 256 local projection dims per core
SCALE = 1.0 / np.sqrt(np.sqrt(float(DK)))  # fold 1/sqrt(dk) half into Q, half into K

_cache: dict = {}


def _build():
    import concourse.mybir as mybir
    import concourse.tile as tile
    from concourse import bacc

    dt = mybir.dt
    f32, bf16 = dt.float32, dt.bfloat16

    nc = bacc.Bacc("TRN2", target_bir_lowering=False, debug=False,
                   num_devices=N_CORES)

    xqT = nc.dram_tensor("xqT", [D, S], bf16, kind="ExternalInput").ap()
    xkT = nc.dram_tensor("xkT", [D, S], bf16, kind="ExternalInput").ap()
    xvT = nc.dram_tensor("xvT", [D, S], bf16, kind="ExternalInput").ap()
    wqT = nc.dram_tensor("wqT", [D, DL], bf16, kind="ExternalInput").ap()
    wkT = nc.dram_tensor("wkT", [D, DL], bf16, kind="ExternalInput").ap()
    wvT = nc.dram_tensor("wvT", [D, DL], bf16, kind="ExternalInput").ap()
    woT = nc.dram_tensor("woT", [DL, D], bf16, kind="ExternalInput").ap()
    bqk = nc.dram_tensor("bqk", [2, DL], f32, kind="ExternalInput").ap()
    out = nc.dram_tensor("out", [S, D], f32, kind="ExternalOutput").ap()

    EXPF = mybir.ActivationFunctionType.Exp

    with tile.TileContext(nc) as tc:
        with (
            tc.tile_pool(name="res", bufs=1) as res,
            tc.tile_pool(name="wts", bufs=1) as wts,
            tc.tile_pool(name="xin", bufs=2) as xin,
            tc.tile_pool(name="expp", bufs=3) as expp,
            tc.tile_pool(name="nrm", bufs=2) as nrm,
            tc.tile_pool(name="drm", bufs=2, space="DRAM") as drm,
            tc.tile_pool(name="ps_proj", bufs=2, space="PSUM") as ps_proj,
            tc.tile_pool(name="ps_st", bufs=3, space="PSUM") as ps_st,
        ):
            # ---- resident tensors ----
            QT = [res.tile([128, S], bf16, name=f"QT{m}", tag=f"QT{m}")
                  for m in range(2)]
            KT = [res.tile([128, S], bf16, name=f"KT{m}", tag=f"KT{m}")
                  for m in range(2)]
            Vsb = res.tile([128, 16, 4, DK + 1], bf16)
            ATT = [res.tile([128, S], bf16, name=f"ATT{m}", tag=f"ATT{m}")
                   for m in range(2)]

            wq_sb = wts.tile([128, 8, DL], bf16, tag="wq")
            wk_sb = wts.tile([128, 8, DL], bf16, tag="wk")
            wv_sb = wts.tile([128, 8, DL], bf16, tag="wv")
            wo_sb = wts.tile([128, 2, D], bf16, tag="wo")
            b_sb = wts.tile([128, 2, 2], f32, tag="b")  # [p, proj(q/k), m]

            for kt in range(8):
                nc.sync.dma_start(wq_sb[:, kt, :], wqT[kt * 128:(kt + 1) * 128, :])
                nc.sync.dma_start(wk_sb[:, kt, :], wkT[kt * 128:(kt + 1) * 128, :])
                nc.sync.dma_start(wv_sb[:, kt, :], wvT[kt * 128:(kt + 1) * 128, :])
            for m in range(2):
                nc.sync.dma_start(wo_sb[:, m, :], woT[m * 128:(m + 1) * 128, :])
                for pj in range(2):
                    nc.sync.dma_start(b_sb[:, pj, m:m + 1],
                                      bqk[pj, m * 128:(m + 1) * 128, None])
            # ones column for softmax denominators
            nc.vector.memset(Vsb[:, :, :, DK], 1.0)

            # ---- Phase B: V projection -> Vsb [kpos, ktile, head, dk] ----
            for stq in range(4):
                xb = xin.tile([128, 8, 512], bf16, tag="xblk")
                nc.sync.dma_start(
                    xb[:, :, :],
                    xvT[:, stq * 512:(stq + 1) * 512].rearrange(
                        "(n p) d -> p n d", p=128))
                for sts in range(4):
                    st = stq * 4 + sts
                    ps = ps_proj.tile([128, 4, DK], f32, tag="proj")
                    for kt in range(8):
                        nc.tensor.matmul(
                            ps[:, :, :],
                            lhsT=xb[:, kt, sts * 128:(sts + 1) * 128],
                            rhs=wv_sb[:, kt, :],
                            start=(kt == 0), stop=(kt == 7))
                    nc.vector.tensor_copy(Vsb[:, st, :, 0:DK], ps[:, :, :])

            # ---- Phase A: Q, K projections -> QT/KT [dout, s] ----
            for m in range(2):
                for pj, (xsrc, wsb, dst) in enumerate(
                        [(xqT, wq_sb, QT), (xkT, wk_sb, KT)]):
                    for sb in range(4):
                        xb = xin.tile([128, 8, 512], bf16, tag="xblk",
                                      name=f"xb{m}{pj}{sb}")
                        nc.sync.dma_start(
                            xb[:, :, :],
                            xsrc[:, sb * 512:(sb + 1) * 512].rearrange(
                                "(n p) d -> p n d", p=128))
                        ps = ps_proj.tile([128, 512], f32, tag="proj")
                        for kt in range(8):
                            nc.tensor.matmul(
                                ps[:, :],
                                lhsT=wsb[:, kt, m * 128:(m + 1) * 128],
                                rhs=xb[:, kt, :],
                                start=(kt == 0), stop=(kt == 7))
                        nc.vector.tensor_scalar_add(
                            dst[m][:, sb * 512:(sb + 1) * 512],
                            ps[:, :], b_sb[:, pj, m:m + 1])

            # ---- Phase C: attention per head pair / q block ----
            for hp in range(2):  # head pair: local heads (2*hp, 2*hp+1)
                for qb in range(4):  # q blocks of 512
                    qs = slice(qb * 512, (qb + 1) * 512)
                    pa = [ps_proj.tile([65, 512], f32, tag="proj",
                                       name=f"at{i}") for i in range(2)]
                    for p in range(8):  # k-tile pairs (kt = 2p, 2p+1)
                        for hh in range(2):  # head within pair
                            lh = 2 * hp + hh
                            lo, hi = hh * 64, hh * 64 + 64
                            st_ps = ps_st.tile([128, 2, 512], f32, tag="st")
                            for j in range(2):
                                kt = 2 * p + j
                                nc.tensor.matmul(
                                    st_ps[:, j, :],
                                    lhsT=KT[hp][lo:hi, kt * 128:(kt + 1) * 128],
                                    rhs=QT[hp][lo:hi, qs],
                                    start=True, stop=True)
                            ex = expp.tile([128, 2, 512], bf16, tag="exp")
                            nc.scalar.activation(ex[:, :, :], st_ps[:, :, :], EXPF)
                            for j in range(2):
                                kt = 2 * p + j
                                nc.tensor.matmul(
                                    pa[hh][:, :],
                                    lhsT=Vsb[:, kt, lh, :],
                                    rhs=ex[:, j, :],
                                    start=(kt == 0), stop=(kt == 15),
                                    skip_group_check=True)
                    # normalize: rows 0..63 / row 64, write into ATT
                    for hh in range(2):
                        asb = nrm.tile([65, 512], f32, tag="asb")
                        nc.vector.tensor_copy(asb[:, :], pa[hh][:, :])
                        # reciprocal of the denom row, spread over 128 lanes
                        rdr = drm.tile([1, 512], f32, tag="rdr")
                        nc.sync.dma_start(rdr[:, :], asb[64:65, :])
                        rq = nrm.tile([128, 4], f32, tag="rq")
                        nc.sync.dma_start(
                            rq[:, :], rdr[0, :].rearrange("(p f) -> p f", p=128))
                        rq2 = nrm.tile([128, 4], f32, tag="rq2")
                        nc.vector.reciprocal(rq2[:, :], rq[:, :])
                        rdr2 = drm.tile([1, 512], f32, tag="rdr2")
                        nc.sync.dma_start(
                            rdr2[0, :].rearrange("(p f) -> p f", p=128),
                            rq2[:, :])
                        rb = nrm.tile([64, 512], f32, tag="rb")
                        nc.sync.dma_start(rb[:, :],
                                          rdr2[:, :].to_broadcast((64, 512)))
                        if hh == 0:
                            nc.vector.tensor_mul(ATT[hp][0:64, qs],
                                                 asb[0:64, :], rb[:, :])
                        else:
                            tmp = nrm.tile([64, 512], bf16, tag="tmp")
                            nc.vector.tensor_mul(tmp[:, :], asb[0:64, :],
                                                 rb[:, :])
                            nc.sync.dma_start(ATT[hp][64:128, qs], tmp[:, :])

            # ---- Phase D: output projection (partial) ----
            for st in range(16):
                for db in range(2):
                    ps = ps_proj.tile([128, 512], f32, tag="proj")
                    for m in range(2):
                        nc.tensor.matmul(
                            ps[:, :],
                            lhsT=ATT[m][:, st * 128:(st + 1) * 128],
                            rhs=wo_sb[:, m, db * 512:(db + 1) * 512],
                            start=(m == 0), stop=(m == 1))
                    osb = nrm.tile([128, 512], f32, tag="osb")
                    nc.vector.tensor_copy(osb[:, :], ps[:, :])
                    nc.sync.dma_start(
                        out[st * 128:(st + 1) * 128, db * 512:(db + 1) * 512],
                        osb[:, :])

    nc.compile()
    return nc


def _prep_inputs(q, k, v, wq, bq, wk, bk, wv, bv, wo, bo):
    q, k, v = (np.asarray(a, np.float32) for a in (q, k, v))
    wq, bq, wk, bk, wv, bv, wo, bo = (
        np.asarray(a, np.float32) for a in (wq, bq, wk, bk, wv, bv, wo, bo))

    xT = {}
    for b in range(B):
        xT[("q", b)] = np.ascontiguousarray(q[b].T).astype(BF16)
        xT[("k", b)] = np.ascontiguousarray(k[b].T).astype(BF16)
        xT[("v", b)] = np.ascontiguousarray(v[b].T).astype(BF16)

    grp = {}
    for g in range(GROUPS):
        hs = slice(g * DL, (g + 1) * DL)
        grp[g] = {
            "wqT": np.ascontiguousarray((wq[hs, :] * SCALE).T).astype(BF16),
            "wkT": np.ascontiguousarray((wk[hs, :] * SCALE).T).astype(BF16),
            "wvT": np.ascontiguousarray(wv[hs, :].T).astype(BF16),
            "woT": np.ascontiguousarray(wo[:, hs].T).astype(BF16),
            "bqk": np.stack([bq[hs] * SCALE, bk[hs] * SCALE]).astype(np.float32),
        }

    in_maps = []
    for c in range(N_CORES):
        b, g = c // GROUPS, c % GROUPS
        m = {"xqT": xT[("q", b)], "xkT": xT[("k", b)], "xvT": xT[("v", b)]}
        m.update(grp[g])
        in_maps.append(m)

    const = (bv @ wo.T + bo).astype(np.float32)  # exact since sum(P) == 1
    return in_maps, const


def _run(in_maps, trace=False):
    from concourse.bass_utils import run_bass_kernel_spmd

    if "nc" not in _cache:
        _cache["nc"] = _build()
    return run_bass_kernel_spmd(_cache["nc"], in_maps, list(range(N_CORES)),
                                trace=trace)


def _reduce(results, const):
    out = np.zeros((B, S, D), np.float32)
    for c in range(N_CORES):
        out[c // GROUPS] += results[c]["out"]
    out += const
    return out


def kernel(**inputs) -> np.ndarray:
    in_maps, const = _prep_inputs(**inputs)
    res = _run(in_maps, trace=False)
    return _reduce(res.results, const)


def kernel_profiled(**inputs):
    """Returns (output, exec_time_ns or None)."""
    in_maps, const = _prep_inputs(**inputs)
    res = _run(in_maps, trace=True)
    return _reduce(res.results, const), res.exec_time_ns
